# revision 11
# baseline (speedup 1.0000x reference)
"""Trainium2 Bass kernel for GQA attention with RoPE (dense_transformer).

Model: B=2, T=2048, C=2048, H=16 query heads, KV=4 kv heads, D=128, causal.
Sharding: 8 cores = batch(2) x kv-group(4) tensor parallel. Each core computes
its batch's 4 query heads (one kv head), then a partial output projection over
its 512 head-dims; per-q-group ReduceScatters (4 cores per batch) sum the
partials. The host reassembles the stripes.

Fused pipeline: for each 512-token chunk tc, project Q/K/V for that chunk,
then immediately run attention for query group tc (its key/value prefix is
complete), then the output projection + ReduceScatter for that group. This
keeps the tensor engine saturated from ~15us in and spreads the collectives
across the whole kernel instead of bunching them at the end.

Layout notes:
 - All projections run with the contraction dim (C) on SBUF partitions; x and
   the weights are transposed on the tensor engine at load time (bf16).
 - wq/wk rows are permuted even/odd at DMA time so RoPE operates on
   partition-aligned halves; scores are invariant to the shared permutation.
 - V is projected directly into [t, dv] layout (lhsT=xT chunk, rhs=wvT), so
   no separate V transpose pass is needed.
 - Scores are computed transposed ([tk, tq]) so softmax'd probabilities feed
   the PV matmul directly; softmax denominators come from a ones-vector
   matmul and the normalization is a broadcast (K=1 matmul) + multiply.
 - PSUM rings (8 banks): ptr(1, bf16 transposes) acc(2, QKV/V/WO accum)
   pscore(2) pattn(2) psums+pbc(1, shared tag ring).
"""

import os

os.environ.setdefault("MYCRO_LOCAL_CACHE", "1")

import numpy as np

B, T, C = 2, 2048, 2048
H, KV, D = 16, 4, 128
HL = H // KV          # 4 local query heads per core
NCORES = 8
P = 128
SCALE = 1.0 / float(np.sqrt(D))

NT = T // P           # 16 t-blocks
NCC = C // P          # 16 c-chunks
NTC = T // 512        # 4 t-chunks of 512
NG = T // 512         # 4 q-block groups (512 queries each)
TQ = 512              # queries per attention group
NEG = -1e10


def _emit(nc, tile, mybir, ExitStack):
    from concourse.masks import make_identity

    f32 = mybir.dt.float32
    bf16 = mybir.dt.bfloat16
    Exp = mybir.ActivationFunctionType.Exp
    Copy = mybir.ActivationFunctionType.Copy
    add = mybir.AluOpType.add

    xb = nc.dram_tensor("xb", [T, C], f32, kind="ExternalInput")
    wq = nc.dram_tensor("wq", [HL * D, C], f32, kind="ExternalInput")
    wk = nc.dram_tensor("wk", [D, C], f32, kind="ExternalInput")
    wv = nc.dram_tensor("wv", [D, C], f32, kind="ExternalInput")
    wo = nc.dram_tensor("wo", [C, HL * D], f32, kind="ExternalInput")
    fcos = nc.dram_tensor("fcos", [T, D // 2], f32, kind="ExternalInput")
    fsin = nc.dram_tensor("fsin", [T, D // 2], f32, kind="ExternalInput")
    out = nc.dram_tensor("out", [T // 4, C], bf16, kind="ExternalOutput")

    te, ve, sc, gp, sy = nc.tensor, nc.vector, nc.scalar, nc.gpsimd, nc.sync

    with tile.TileContext(nc) as tc, ExitStack() as ctx:
        consts = ctx.enter_context(tc.tile_pool(name="consts", bufs=1))
        persist = ctx.enter_context(tc.tile_pool(name="persist", bufs=1))
        dram = ctx.enter_context(tc.tile_pool(name="dram", bufs=1, space="DRAM"))

        sbIn = ctx.enter_context(tc.tile_pool(name="sbIn", bufs=2))
        sbBf = ctx.enter_context(tc.tile_pool(name="sbBf", bufs=2))
        sbWi = ctx.enter_context(tc.tile_pool(name="sbWi", bufs=3))
        sbWb = ctx.enter_context(tc.tile_pool(name="sbWb", bufs=4))
        sbQK = ctx.enter_context(tc.tile_pool(name="sbQK", bufs=3))
        sbR = ctx.enter_context(tc.tile_pool(name="sbR", bufs=2))
        sbP = ctx.enter_context(tc.tile_pool(name="sbP", bufs=6))
        sbC = ctx.enter_context(tc.tile_pool(name="sbC", bufs=2))
        sbBC = ctx.enter_context(tc.tile_pool(name="sbBC", bufs=2))
        sbD = ctx.enter_context(tc.tile_pool(name="sbD", bufs=1))
        sbQR = ctx.enter_context(tc.tile_pool(name="sbQR", bufs=2))
        sbAT = ctx.enter_context(tc.tile_pool(name="sbAT", bufs=2))

        psTr = ctx.enter_context(tc.tile_pool(name="psTr", bufs=1, space="PSUM"))
        psAcc = ctx.enter_context(tc.tile_pool(name="psAcc", bufs=2, space="PSUM"))
        psSc = ctx.enter_context(tc.tile_pool(name="psSc", bufs=2, space="PSUM"))
        psAt = ctx.enter_context(tc.tile_pool(name="psAt", bufs=2, space="PSUM"))
        psSm = ctx.enter_context(tc.tile_pool(name="psSm", bufs=1, space="PSUM"))

        ident = consts.tile([P, P], bf16, tag="ident")
        make_identity(nc, ident[:])
        identf = consts.tile([P, P], f32, tag="identf")
        make_identity(nc, identf[:])
        # scoresT layout [tk, tq]: keep where tq >= tk, else -1e10.
        triT = consts.tile([P, P], f32, tag="triT")
        gp.memset(triT[:], 0.0)
        gp.affine_select(
            out=triT[:], in_=triT[:], compare_op=mybir.AluOpType.is_ge,
            fill=NEG, base=0, pattern=[[1, P]], channel_multiplier=-1,
        )
        ones = consts.tile([P, 1], bf16, tag="ones")
        gp.memset(ones[:], 1.0)
        onesc = consts.tile([P, P], bf16, tag="onesc")
        gp.memset(onesc[:], 1.0)

        # persistent per-core state
        cosq = persist.tile([P, T], bf16, tag="cosq")
        sinq = persist.tile([P, T], bf16, tag="sinq")
        cosk = persist.tile([P, T], bf16, tag="cosk")
        sink = persist.tile([P, T], bf16, tag="sink")
        wqT = [persist.tile([P, HL * P], bf16, tag=f"wqT{cc}", name=f"wqT{cc}")
               for cc in range(NCC)]
        wkT = [persist.tile([P, P], bf16, tag=f"wkT{cc}", name=f"wkT{cc}")
               for cc in range(NCC)]
        wvT = [persist.tile([P, P], bf16, tag=f"wvT{cc}", name=f"wvT{cc}")
               for cc in range(NCC)]
        woT = [persist.tile([P, C], bf16, tag=f"woT{h}", name=f"woT{h}")
               for h in range(HL)]
        krT = persist.tile([P, T], bf16, tag="krT")
        vnat = persist.tile([P, T], bf16, tag="vnat")
        xT = [persist.tile([P, 512], bf16, tag=f"xT{cc}", name=f"xT{cc}")
              for cc in range(NCC)]

        y_dram = [dram.tile([TQ, C], bf16, tag=f"ydram{g}", name=f"ydram{g}")
                  for g in range(NG)]
        rs_out = [dram.tile([64, C], bf16, tag=f"rsout{g}", name=f"rsout{g}")
                  for g in range(2 * NG)]

        # ---- startup DMAs -------------------------------------------------
        # x chunk 0 (sy/sc), wq/wk/wv (gp), freqs (sc)
        def load_x_chunk(tc4):
            """DMA 4 row-blocks of x and convert to bf16 (in 512-col pieces
            so the vector engine can interleave attention work)."""
            xbf = []
            for i in range(4):
                tb = tc4 * 4 + i
                xt = sbIn.tile([P, C], f32, tag="big_in", name="big_in")
                (sy if tb % 2 == 0 else sc).dma_start(
                    xt[:], xb.ap()[tb * P:(tb + 1) * P, :])
                xc = sbBf.tile([P, C], bf16, tag="big_bf", name="big_bf",
                               bufs=4)
                for j in range(4):
                    ve.tensor_copy(xc[:, j * 512:(j + 1) * 512],
                                   xt[:, j * 512:(j + 1) * 512])
                xbf.append(xc)
            return xbf

        xbf_next = load_x_chunk(0)

        # wq: load+convert+transpose one head at a time (keeps only one
        # [P, C] tile alive); transposes write per-head column stripes.
        wq_eo = wq.ap().rearrange("(a two) c -> two a c", two=2)
        for h in range(HL):
            wt = sbIn.tile([P, C], f32, tag="wq_in", name="wq_in", bufs=1)
            gp.dma_start(wt[0:64, :], wq_eo[0, h * 64:(h + 1) * 64, :])
            gp.dma_start(wt[64:P, :], wq_eo[1, h * 64:(h + 1) * 64, :])
            wb = sbBf.tile([P, C], bf16, tag="wq_bf", name="wq_bf")
            for j in range(4):
                sc.activation(wb[:, j * 512:(j + 1) * 512],
                              wt[:, j * 512:(j + 1) * 512], Copy)
            for ccg in range(NCC // 4):
                pt = psTr.tile([P, 512], bf16, tag="ptr", name="ptr")
                for i in range(4):
                    cc = ccg * 4 + i
                    te.transpose(pt[:, i * P:(i + 1) * P],
                                 wb[:, cc * P:(cc + 1) * P], ident[:])
                for i in range(4):
                    ve.tensor_copy(wqT[ccg * 4 + i][:, h * P:(h + 1) * P],
                                   pt[:, i * P:(i + 1) * P])

        wk_eo = wk.ap().rearrange("(a two) c -> two a c", two=2)
        for src_eo, src, dst, perm in (
                (wk_eo, wk, wkT, True), (None, wv, wvT, False)):
            wt = sbIn.tile([P, C], f32, tag="wkv_in", name="wkv_in", bufs=1)
            if perm:
                gp.dma_start(wt[0:64, :], src_eo[0, :, :])
                gp.dma_start(wt[64:P, :], src_eo[1, :, :])
            else:
                gp.dma_start(wt[:], src.ap()[:, :])
            wb = sbBf.tile([P, C], bf16, tag="wkv_bf", name="wkv_bf")
            for j in range(4):
                ve.tensor_copy(wb[:, j * 512:(j + 1) * 512],
                               wt[:, j * 512:(j + 1) * 512])
            for ccg in range(NCC // 4):
                pt = psTr.tile([P, 512], bf16, tag="ptr", name="ptr")
                for i in range(4):
                    cc = ccg * 4 + i
                    te.transpose(pt[:, i * P:(i + 1) * P],
                                 wb[:, cc * P:(cc + 1) * P], ident[:])
                for i in range(4):
                    sc.activation(dst[ccg * 4 + i][:],
                                  pt[:, i * P:(i + 1) * P], Copy)

        # freqs -> cos/sin tables [128, T] bf16 (halves duplicated so rope
        # reads stay partition-base-aligned); q copies pre-scaled. f32
        # transposes ride the pscore ring (free until first attention).
        for src, dq, dk in ((fcos, cosq, cosk), (fsin, sinq, sink)):
            for tb in range(NT):
                ft = sbIn.tile([P, 64], f32, tag="frq_in", name="frq_in",
                               bufs=3)
                sc.dma_start(ft[:], src.ap()[tb * P:(tb + 1) * P, :])
                pf = psSc.tile([P, TQ], f32, tag="pscore", name="pscore")
                te.transpose(pf[0:64, 0:P], ft[:], identf[:])
                sc.activation(dq[0:64, tb * P:(tb + 1) * P], pf[0:64, 0:P],
                              Copy, scale=SCALE)
                sc.activation(dk[0:64, tb * P:(tb + 1) * P], pf[0:64, 0:P],
                              Copy)
            sy.dma_start(dq[64:P, :], dq[0:64, :])
            sy.dma_start(dk[64:P, :], dk[0:64, :])

        # ---- helpers ------------------------------------------------------
        def rope(dst, dsl, qs, cos_t, sin_t, gsl):
            """RoPE for one [128(d, eo-permuted), 512(t)] tile. dst gets the
            rotated values in columns dsl; cos/sin read global gsl."""
            q1lo = sbR.tile([64, 512], bf16, tag="q1lo", name="q1lo")
            sy.dma_start(q1lo[:], qs[64:P, :])
            q0hi = sbR.tile([P, 512], bf16, tag="q0hi", name="q0hi")
            sy.dma_start(q0hi[64:P, :], qs[0:64, :])
            tb2 = sbR.tile([64, 512], bf16, tag="rtb", name="rtb")
            ve.tensor_mul(dst[0:64, dsl], qs[0:64, :], cos_t[0:64, gsl])
            ve.tensor_mul(tb2[:], q1lo[:], sin_t[0:64, gsl])
            ve.tensor_sub(dst[0:64, dsl], dst[0:64, dsl], tb2[:])
            tc2 = sbR.tile([P, 512], bf16, tag="rtc", name="rtc")
            ve.tensor_mul(dst[64:P, dsl], qs[64:P, :], cos_t[64:P, gsl])
            ve.tensor_mul(tc2[64:P, :], q0hi[64:P, :], sin_t[64:P, gsl])
            ve.tensor_add(dst[64:P, dsl], dst[64:P, dsl], tc2[64:P, :])

        def emit_scores(gq, kb, hs, qrT):
            """scoresT + exp for one k-block, two heads; returns probs."""
            j = kb - 4 * gq
            w0 = max(j, 0) * P
            probs = []
            for h in hs:
                st = psSc.tile([P, TQ], f32, tag="pscore", name="pscore")
                te.matmul(
                    st[:, w0:TQ],
                    krT[:, kb * P:(kb + 1) * P],
                    qrT[h][:, w0:TQ],
                    start=True, stop=True,
                )
                if j >= 0:
                    ve.tensor_tensor(
                        st[:, w0:w0 + P], st[:, w0:w0 + P], triT[:], add)
                pb = sbP.tile([P, TQ], bf16, tag="probs", name="probs")
                sc.activation(pb[:, w0:TQ], st[:, w0:TQ], Exp)
                probs.append(pb)
            return probs, w0

        def emit_accum(kb, kbmax, w0, probs, pa, psums):
            for i in range(2):
                te.matmul(
                    psums[64 * i:64 * i + 1, w0:TQ], ones[:],
                    probs[i][:, w0:TQ],
                    start=(kb == 0), stop=(kb == kbmax - 1),
                )
            for i in range(2):
                te.matmul(
                    pa[i][:, w0:TQ], vnat[:, kb * P:(kb + 1) * P],
                    probs[i][:, w0:TQ],
                    start=(kb == 0), stop=(kb == kbmax - 1),
                )

        def emit_wo_prep():
            """wo [C, HL*D] -> woT[h] [dv, C]."""
            for ctg in range(NCC // 4):
                wo_bf = []
                for i in range(4):
                    ct = ctg * 4 + i
                    wt = sbWi.tile([P, HL * P], f32, tag="wo_in", name="wo_in")
                    gp.dma_start(wt[:], wo.ap()[ct * P:(ct + 1) * P, :])
                    wb = sbWb.tile([P, HL * P], bf16, tag="wo_bf",
                                   name="wo_bf")
                    sc.activation(wb[:], wt[:], Copy)
                    wo_bf.append(wb)
                for h in range(HL):
                    pt = psTr.tile([P, 512], bf16, tag="ptr", name="ptr")
                    for i in range(4):
                        te.transpose(pt[:, i * P:(i + 1) * P],
                                     wo_bf[i][:, h * P:(h + 1) * P], ident[:])
                    sc.activation(woT[h][:, ctg * 512:(ctg + 1) * 512],
                                  pt[:], Copy)

        def emit_wo_group(gq, attnT):
            """output projection for group gq's 4 t-blocks + reduce-scatter"""
            for tb4 in range(4):
                ysb = sbD.tile([P, C], bf16, tag="ysb", name="ysb")
                tb0 = tb4 * P
                for cc4 in range(C // 512):
                    py = psAcc.tile([P, 512], f32, tag="acc", name="acc")
                    for h in range(HL):
                        te.matmul(
                            py[:],
                            attnT[h][:, tb0:tb0 + P],
                            woT[h][:, cc4 * 512:(cc4 + 1) * 512],
                            start=(h == 0), stop=(h == HL - 1),
                        )
                    sc.activation(ysb[:, cc4 * 512:(cc4 + 1) * 512], py[:],
                                  Copy)
                sy.dma_start(y_dram[gq][tb4 * P:(tb4 + 1) * P, :], ysb[:])
                if tb4 % 2 == 1:
                    hf = tb4 // 2
                    gp.collective_compute(
                        "ReduceScatter", mybir.AluOpType.add,
                        replica_groups=[[0, 1, 2, 3], [4, 5, 6, 7]],
                        ins=[y_dram[gq][256 * hf:256 * (hf + 1), :].opt()],
                        outs=[rs_out[2 * gq + hf].opt()],
                    )
                    sy.dma_start(
                        out.ap()[gq * P + 64 * hf:gq * P + 64 * (hf + 1), :],
                        rs_out[2 * gq + hf][:])

        # ---- fused per-chunk pipeline ------------------------------------
        pend_wo = None  # (gq, attnT) whose output projection is deferred
        for tc4 in range(NTC):
            gsl = slice(tc4 * 512, (tc4 + 1) * 512)

            # x transposes for this chunk
            xbf = xbf_next
            for cc in range(NCC):
                pt = psTr.tile([P, 512], bf16, tag="ptr", name="ptr")
                for i in range(4):
                    te.transpose(pt[:, i * P:(i + 1) * P],
                                 xbf[i][:, cc * P:(cc + 1) * P], ident[:])
                ve.tensor_copy(xT[cc][:], pt[:])

            # Q/K projections for this chunk (+rope)
            qrT = [sbQR.tile([P, 512], bf16, tag=f"qrT{h}", name=f"qrT{h}")
                   for h in range(HL)]
            for h in range(HL):
                ps = psAcc.tile([P, 512], f32, tag="acc", name="acc")
                for cc in range(NCC):
                    te.matmul(ps[:], wqT[cc][:, h * P:(h + 1) * P], xT[cc][:],
                              start=(cc == 0), stop=(cc == NCC - 1))
                qs = sbQK.tile([P, 512], bf16, tag="qkev", name="qkev")
                sc.activation(qs[:], ps[:], Copy)
                rope(qrT[h], slice(0, 512), qs, cosq, sinq, gsl)
            ps = psAcc.tile([P, 512], f32, tag="acc", name="acc")
            for cc in range(NCC):
                te.matmul(ps[:], wkT[cc][:], xT[cc][:],
                          start=(cc == 0), stop=(cc == NCC - 1))
            qs = sbQK.tile([P, 512], bf16, tag="qkev", name="qkev")
            sc.activation(qs[:], ps[:], Copy)
            rope(krT, gsl, qs, cosk, sink, gsl)

            # V directly in [t, dv] layout: lhsT = xT block, rhs = wvT
            ps = psAcc.tile([P, 512], f32, tag="acc", name="acc")
            for tb4 in range(4):
                cs = slice(tb4 * P, (tb4 + 1) * P)
                for cc in range(NCC):
                    te.matmul(ps[:, cs], xT[cc][:, cs], wvT[cc][:],
                              start=(cc == 0), stop=(cc == NCC - 1))
            sc.activation(vnat[:, gsl], ps[:], Copy)

            # deferred output projection from the previous group (kept out
            # of chunk 0 so startup DMA/conversions aren't crowded)
            if tc4 == 1:
                emit_wo_prep()
            if pend_wo is not None:
                emit_wo_group(*pend_wo)
                pend_wo = None

            # attention for query group tc4
            gq = tc4
            kbmax = 4 * (gq + 1)
            attnT = [sbAT.tile([P, TQ], bf16, tag=f"attnT{h}",
                               name=f"attnT{h}") for h in range(HL)]
            for hp in range(HL // 2):
                hs = (2 * hp, 2 * hp + 1)
                pa = [psAt.tile([P, TQ], f32, tag="pattn", name="pattn")
                      for _ in hs]
                psums = psSm.tile([P, TQ], f32, tag="psums", name="psums")
                # software-pipelined: scores(kb+1) issue before accum(kb)
                prev = None
                for kb in range(kbmax):
                    cur = (kb, *emit_scores(gq, kb, hs, qrT))
                    if prev is not None:
                        pkb, pprobs, pw0 = prev
                        emit_accum(pkb, kbmax, pw0, pprobs, pa, psums)
                    prev = cur
                pkb, pprobs, pw0 = prev
                emit_accum(pkb, kbmax, pw0, pprobs, pa, psums)

                # evict unnormalized; normalize off the critical path
                sums_sb = sbC.tile([P, TQ], f32, tag="sums_sb",
                                   name="sums_sb", bufs=1)
                sc.activation(sums_sb[0:1, :], psums[0:1, :], Copy)
                sc.activation(sums_sb[64:65, :], psums[64:65, :], Copy)
                for i, h in enumerate(hs):
                    sc.activation(attnT[h][:], pa[i][:], Copy)
                recip = sbC.tile([P, TQ], bf16, tag="recip", name="recip", bufs=1)
                with nc.allow_low_precision(reason="softmax recip bf16"):
                    ve.reciprocal(recip[0:1, :], sums_sb[0:1, :])
                    ve.reciprocal(recip[64:65, :], sums_sb[64:65, :])
                for i, h in enumerate(hs):
                    pbc = psSm.tile([P, TQ], f32, tag="psums", name="psums")
                    te.matmul(pbc[:], onesc[64 * i:64 * i + 1, 0:P],
                              recip[64 * i:64 * i + 1, :],
                              start=True, stop=True)
                    bc = sbBC.tile([P, TQ], bf16, tag="rbc", name="rbc")
                    sc.activation(bc[:], pbc[:], Copy)
                    ve.tensor_mul(attnT[h][:], attnT[h][:], bc[:])

            # prefetch next chunk's x (emitted after attention so the
            # conversions don't crowd the attention-critical vector ops)
            if tc4 + 1 < NTC:
                xbf_next = load_x_chunk(tc4 + 1)

            if tc4 == NTC - 1:
                emit_wo_group(gq, attnT)
            else:
                pend_wo = (gq, attnT)

    return nc


_PROGRAM = None


def _get_program():
    global _PROGRAM
    if _PROGRAM is None:
        from contextlib import ExitStack
        import concourse.tile as tile
        from concourse import bacc, mybir

        nc = bacc.Bacc("TRN2", target_bir_lowering=False, debug=False,
                       num_devices=NCORES)
        _emit(nc, tile, mybir, ExitStack)
        nc.compile()
        _PROGRAM = nc
    return _PROGRAM


def kernel(x, wq, wk, wv, wo, freqs_cos, freqs_sin, mask=None):
    from concourse.bass_utils import run_bass_kernel_spmd

    x = np.asarray(x, np.float32)
    wq = np.asarray(wq, np.float32)
    wk = np.asarray(wk, np.float32)
    wv = np.asarray(wv, np.float32)
    wo = np.asarray(wo, np.float32)
    fc = np.ascontiguousarray(np.asarray(freqs_cos, np.float32))
    fs = np.ascontiguousarray(np.asarray(freqs_sin, np.float32))

    nc = _get_program()
    in_maps = []
    for core in range(NCORES):
        b, g = core // 4, core % 4
        in_maps.append({
            "xb": np.ascontiguousarray(x[b]),
            "wq": np.ascontiguousarray(wq[g * HL * D:(g + 1) * HL * D]),
            "wk": np.ascontiguousarray(wk[g * D:(g + 1) * D]),
            "wv": np.ascontiguousarray(wv[g * D:(g + 1) * D]),
            "wo": np.ascontiguousarray(wo[:, g * HL * D:(g + 1) * HL * D]),
            "fcos": fc,
            "fsin": fs,
        })
    res = run_bass_kernel_spmd(nc, in_maps, core_ids=list(range(NCORES)))
    outp = np.empty((B, T, C), np.float32)
    for b in range(B):
        for r in range(4):
            piece = np.asarray(res.results[4 * b + r]["out"],
                               dtype=np.float32)  # [NG*128, C]
            for gq in range(NG):
                for hf in range(2):
                    dst = 512 * gq + 256 * hf + 64 * r
                    srow = 128 * gq + 64 * hf
                    outp[b, dst:dst + 64] = piece[srow:srow + 64]
    return outp
